# revision 45
# baseline (speedup 1.0000x reference)
"""MHA (B=2, S=2048, D=1024, H=16) on 8 Trainium2 NeuronCores - bf16 edition.

Sharding: core c = (batch b = c//4, head-group g = c%4, 4 heads each).

All matmuls in bf16 (same PE rate as fp32r, half the DMA/SBUF traffic;
fp8 DoubleRow was tried and rejected: the sharp softmax rows amplify
any ~2% quantization noise past the accuracy gate). Structural wins
over the fp32r baseline:
 - softmax exp is split ACT(24/32, exact Exp) + DVE(8/32, Schraudolph
   fast-exp emitted as bf16 *bits* via a saturating fp32->uint16
   tensor_scalar), so the Activation engine is no longer the
   attention bottleneck; attention is PE-bound.
 - three AllToAlls (heads 0+1, head 2, head 3) in bf16 instead of four
   fp32 ones, mostly hidden under attention compute.
 - the out-projection runs in three waves (even k-tiles, then the
   head-2 and head-3 halves of the odd k-tiles as K=64 matmuls), each
   gated by one collective, so only the last wave sits on the tail.
 - scores PSUM is four 512-column banks deep so the PE never exposes
   the matmul->exp handoff latency; attention@V streams three
   key-blocks behind the exp wave.
"""

import numpy as np
import ml_dtypes

B, S, D, H = 2, 2048, 1024, 16
HD = D // H          # 64
GH = 4               # heads per core
CORES = 8
QS = S // CORES      # 256 per-core final sequence slice (per batch)
P = 128
KT = D // P          # 8 k-tiles of the model dim
VW = HD + 1          # 65: per-head V width incl. ones column

A16 = 128.0 / float(np.log(2.0))   # schraudolph slope (bf16-bit domain)
B16 = 16255.35                     # schraudolph offset (tuned)
LNC = 0.03638                      # ln(c): ACT path matches schraudolph scale

_CACHE = {}


def _exp_engine_pattern():
    """Weighted round-robin over (ACT, DVE) for the 64 exp tiles of one head.
    ACT-heavy keeps the approximate-exp share small; attention stays PE-bound
    either way."""
    shares = {"A": 40, "D": 24}
    counts = {k: 0 for k in shares}
    seq = []
    for i in range(64):
        pick = max(shares, key=lambda k: shares[k] * (i + 1) / 64 - counts[k])
        counts[pick] += 1
        seq.append(pick)
    return seq


def _build_nc():
    import concourse.mybir as mybir
    import concourse.tile as tile
    from concourse import bacc

    F32 = mybir.dt.float32
    BF16 = mybir.dt.bfloat16
    U16 = mybir.dt.uint16
    EXP = mybir.ActivationFunctionType.Exp
    IDN = mybir.ActivationFunctionType.Identity
    MUL = mybir.AluOpType.mult
    ADD = mybir.AluOpType.add

    nc = bacc.Bacc("TRN2", target_bir_lowering=False, debug=False,
                   num_devices=CORES)

    d_x = nc.dram_tensor("x16", [D, S], BF16, kind="ExternalInput")
    d_wq = nc.dram_tensor("wq16", [D, 256], BF16, kind="ExternalInput")
    d_wk = nc.dram_tensor("wk16", [D, 256], BF16, kind="ExternalInput")
    d_wv = nc.dram_tensor("wv16", [D, 256], BF16, kind="ExternalInput")
    d_wo = nc.dram_tensor("wo16", [D, D], BF16, kind="ExternalInput")
    d_bq = nc.dram_tensor("bq2", [P, 2], F32, kind="ExternalInput")
    d_bk = nc.dram_tensor("bk2", [P, 2], F32, kind="ExternalInput")
    d_vi = nc.dram_tensor("vib1", [1, 256], F32, kind="ExternalInput")
    d_bo = nc.dram_tensor("bo1", [1, D], F32, kind="ExternalInput")
    d_y = nc.dram_tensor("y", [B, QS, D], F32, kind="ExternalOutput")

    PAT = _exp_engine_pattern()

    with tile.TileContext(nc) as tc:
        with (
            tc.tile_pool(name="statics", bufs=1) as st,
            tc.tile_pool(name="dram", bufs=1, space="DRAM") as dram,
        ):
            bq = st.tile([P, 2], F32, tag="bq", name="bq")
            bk = st.tile([P, 2], F32, tag="bk", name="bk")
            lnc = st.tile([P, 1], F32, tag="lnc", name="lnc")
            vib = st.tile([P, 256], F32, tag="vib", name="vib")
            bob = st.tile([P, D], F32, tag="bob", name="bob")
            nc.vector.memset(lnc[:], LNC)

            # head pair tiles: partition = (h%2)*64 + hd  (PE base 0/64)
            qTp = [st.tile([P, S], BF16, tag=f"qT{m}", name=f"qT{m}")
                   for m in range(2)]
            kTp = [st.tile([P, S], BF16, tag=f"kT{m}", name=f"kT{m}")
                   for m in range(2)]
            vaug = [st.tile([P, GH * VW], BF16, tag=f"va{i}", name=f"va{i}")
                    for i in range(16)]
            pay = [st.tile([HD, S], BF16, tag=f"pay{h}", name=f"pay{h}")
                   for h in range(GH)]
            wo = [st.tile([P, D], BF16, tag=f"wo{k}", name=f"wo{k}")
                  for k in range(KT)]
            # merged A2A readback: aoE/aoO[:, (bb*4+gp)*256 + q] for even/odd
            # k-tiles; one DMA per a_out tensor instead of 24 small ones
            aoE = st.tile([P, 8 * QS], BF16, tag="aoE", name="aoE")
            aoO = st.tile([P, 8 * QS], BF16, tag="aoO", name="aoO")

            # ---- projections ----
            with (
                tc.tile_pool(name="proj", bufs=1) as pr,
                tc.tile_pool(name="pj", bufs=5, space="PSUM") as pj,
                tc.tile_pool(name="pv", bufs=2, space="PSUM") as pvp,
            ):
                wq16 = [pr.tile([P, 256], BF16, tag=f"wq{k}", name=f"wq{k}")
                        for k in range(KT)]
                wk16 = [pr.tile([P, 256], BF16, tag=f"wk{k}", name=f"wk{k}")
                        for k in range(KT)]
                wv16 = [pr.tile([P, 256], BF16, tag=f"wv{k}", name=f"wv{k}")
                        for k in range(KT)]
                xT = [pr.tile([P, S], BF16, tag=f"x{k}", name=f"x{k}")
                      for k in range(KT)]
                vi1 = pr.tile([1, 256], F32, tag="vi1", name="vi1")
                bo1 = pr.tile([1, D], F32, tag="bo1", name="bo1")
                # k-interleaved issue: chain step k waits only on DMAs
                # issued up to (wq,wk,x)[k], so the PE starts ~2us in
                for k in range(KT):
                    nc.sync.dma_start(wq16[k][:], d_wq[k * P:(k + 1) * P, :])
                    nc.sync.dma_start(wk16[k][:], d_wk[k * P:(k + 1) * P, :])
                    nc.sync.dma_start(xT[k][:], d_x[k * P:(k + 1) * P, :])
                nc.sync.dma_start(bq[:], d_bq[:])
                nc.sync.dma_start(bk[:], d_bk[:])
                for i in range(16):
                    ones = vaug[i].rearrange("p (h w) -> p h w", w=VW)
                    nc.vector.memset(ones[:, :, HD:VW], 1.0)

                # Q (bias-copy on ACT) / K (on DVE); k-major within groups of
                # chains so the PE streams behind the x DMA instead of
                # stalling on the full 4MB load; pair 0 first so head 0 can
                # start as soon as possible
                qk_chains = []
                for m in range(2):
                    for nb in range(4):
                        qk_chains.append(("K", m, nb))
                        qk_chains.append(("Q", m, nb))
                for g0 in range(0, 16, 4):
                    grp = qk_chains[g0:g0 + 4]
                    tiles = [pj.tile([P, 512], F32, tag="pj", name="pj")
                             for _ in grp]
                    for k in range(KT):
                        for (pk, m, nb), ps in zip(grp, tiles):
                            w = wk16[k] if pk == "K" else wq16[k]
                            nc.tensor.matmul(
                                ps[:], w[:, m * P:(m + 1) * P],
                                xT[k][:, nb * 512:(nb + 1) * 512],
                                start=(k == 0), stop=(k == KT - 1))
                    for (pk, m, nb), ps in zip(grp, tiles):
                        if pk == "K":
                            nc.vector.tensor_scalar(
                                kTp[m][:, nb * 512:(nb + 1) * 512], ps[:],
                                bk[:, m:m + 1], None, ADD)
                        else:
                            nc.scalar.activation(
                                qTp[m][:, nb * 512:(nb + 1) * 512], ps[:],
                                IDN, bias=bq[:, m:m + 1], scale=1.0)

                # V weights + biases now; wo last (needed only at out-proj)
                for k in range(KT):
                    nc.sync.dma_start(wv16[k][:], d_wv[k * P:(k + 1) * P, :])
                nc.sync.dma_start(vi1[:], d_vi[:])
                nc.sync.dma_start(bo1[:], d_bo[:])
                nc.gpsimd.partition_broadcast(vib[:], vi1[:])
                nc.gpsimd.partition_broadcast(bob[:], bo1[:])
                for k in range(KT):
                    nc.sync.dma_start(wo[k][:], d_wo[k * P:(k + 1) * P, :])

                # V: natural layout [kpos, 4 heads x 64] + ones col
                for sb in range(16):
                    pv = pvp.tile([P, 256], F32, tag="pv", name="pv")
                    for k in range(KT):
                        nc.tensor.matmul(
                            pv[:], xT[k][:, sb * P:(sb + 1) * P], wv16[k][:],
                            start=(k == 0), stop=(k == KT - 1))
                    dst = vaug[sb].rearrange("p (h w) -> p h w", w=VW)
                    nc.vector.tensor_tensor(
                        dst[:, :, 0:HD],
                        pv.rearrange("p (h w) -> p h w", w=HD),
                        vib.rearrange("p (h w) -> p h w", w=HD), ADD)

            # ---- attention ----
            a_ins = [dram.tile([CORES * P, QS], BF16, name="a_in01"),
                     dram.tile([CORES * HD, QS], BF16, name="a_in2"),
                     dram.tile([CORES * HD, QS], BF16, name="a_in3")]
            a_outs = [dram.tile([CORES * P, QS], BF16, name="a_out01"),
                      dram.tile([CORES * HD, QS], BF16, name="a_out2"),
                      dram.tile([CORES * HD, QS], BF16, name="a_out3")]

            def issue_collective(ci, heads):
                for hi, h in enumerate(heads):
                    dst = a_ins[ci].rearrange(
                        "(j r) q -> r j q", j=CORES)[hi * HD:(hi + 1) * HD]
                    src = pay[h].rearrange("p (j q) -> p j q", j=CORES)
                    nc.sync.dma_start(dst, src)
                nc.gpsimd.collective_compute(
                    "AllToAll",
                    mybir.AluOpType.bypass,
                    replica_groups=[list(range(CORES))],
                    ins=[a_ins[ci][:]],
                    outs=[a_outs[ci][:]],
                )

            with (
                tc.tile_pool(name="exp", bufs=1) as exp_pool,
                tc.tile_pool(name="nrm", bufs=2) as nr,
                tc.tile_pool(name="psc", bufs=4, space="PSUM") as psc,
                tc.tile_pool(name="pav", bufs=1, space="PSUM") as pav,
            ):
                for h in range(GH):
                    ksl = kTp[h // 2][(h % 2) * HD:(h % 2 + 1) * HD]
                    qsl = qTp[h // 2][(h % 2) * HD:(h % 2 + 1) * HD]
                    ex = [exp_pool.tile([P, S], BF16, tag=f"ex{i}",
                                        name=f"ex{i}") for i in range(16)]
                    av = pav.tile([VW, S], F32, tag="av", name="av")

                    def av_step(kb):
                        for qb in range(4):
                            nc.tensor.matmul(
                                av[:, qb * 512:(qb + 1) * 512],
                                vaug[kb][:, h * VW:(h + 1) * VW],
                                ex[kb][:, qb * 512:(qb + 1) * 512],
                                start=(kb == 0), stop=(kb == 15))

                    for kb in range(16):
                        # streaming AV three key-blocks behind the exp wave,
                        # issued BEFORE the scores pair so a scores psum-
                        # backpressure stall never head-of-line-blocks it
                        if kb >= 3:
                            av_step(kb - 3)
                        for qt in range(4):
                            sc = psc.tile([P, 512], F32, tag="sc", name="sc")
                            qo = qt * 512
                            nc.tensor.matmul(
                                sc[:], ksl[:, kb * P:(kb + 1) * P],
                                qsl[:, qo:qo + 512],
                                start=True, stop=True)
                            dst = ex[kb][:, qo:qo + 512]
                            if PAT[kb * 4 + qt] == "A":
                                nc.scalar.activation(
                                    dst, sc[:], EXP,
                                    bias=lnc[:, 0:1], scale=1.0)
                            else:
                                nc.vector.tensor_scalar(
                                    dst.bitcast(U16), sc[:],
                                    A16, B16, MUL, ADD)
                    for kb in range(13, 16):
                        av_step(kb)
                    rc = nr.tile([1, S], F32, tag="rc", name="rc")
                    nc.vector.reciprocal(rc[:], av[HD:VW, :])
                    rcb = nr.tile([HD, S], F32, tag="rcb", name="rcb")
                    nc.gpsimd.partition_broadcast(rcb[:], rc[:])
                    nc.vector.tensor_tensor(
                        pay[h][:], av[0:HD, :], rcb[:], MUL)

                    if h == 1:
                        issue_collective(0, [0, 1])
                    elif h == 2:
                        issue_collective(1, [2])
                        # even-k readback here: after this point SP.SEQ waits
                        # on pay[3]; issuing later would delay out-proj even
                        nc.sync.dma_start(
                            aoE.rearrange("p (s q) -> p s q", s=8),
                            a_outs[0].rearrange("(s p) q -> p s q", s=8))
                    elif h == 3:
                        issue_collective(2, [3])
                        # head-2 half of the odd-k readback: a_out2 lands
                        # while collective 3 is in flight
                        nc.sync.dma_start(
                            aoO.rearrange("p (s q) -> p s q", s=8)[0:HD],
                            a_outs[1].rearrange("(s p) q -> p s q", s=8))

            # head-3 half of the odd-k readback (after collective 3)
            nc.sync.dma_start(
                aoO.rearrange("p (s q) -> p s q", s=8)[HD:P],
                a_outs[2].rearrange("(s p) q -> p s q", s=8))

            # ---- out projection (my 256-row slice of each batch) ----
            with (
                tc.tile_pool(name="po", bufs=1, space="PSUM") as po,
                tc.tile_pool(name="yo", bufs=4) as yo,
            ):
                # create tiles in reverse so the first-executed chains sit on
                # the banks the attention scores pool releases earliest
                tiles = {}
                for bb in reversed(range(B)):
                    for m in reversed(range(2)):
                        for n in reversed(range(2)):
                            tiles[(bb, m, n)] = po.tile(
                                [P, 512], F32, tag=f"po{bb}{m}{n}",
                                name=f"po{bb}{m}{n}")
                chains = [(bb, m, n, tiles[(bb, m, n)])
                          for bb in range(B) for m in range(2)
                          for n in range(2)]
                # three waves, each gated by one collective: even k-tiles
                # (heads 0,1), then the head-2 halves of the odd k-tiles
                # (K=64), then the head-3 halves after the last collective
                for phase in range(3):
                    for bb, m, n, ps in chains:
                        for ki in range(4):
                            k = ki * 2 + (1 if phase > 0 else 0)
                            c0 = (bb * 4 + ki) * QS + m * P
                            if phase == 0:
                                src = aoE[:, c0:c0 + P]
                            elif phase == 1:
                                src = aoO[0:HD, c0:c0 + P]
                            else:
                                src = aoO[HD:P, c0:c0 + P]
                            wos = wo[k][:, n * 512:(n + 1) * 512] \
                                if phase == 0 else \
                                wo[k][(phase - 1) * HD:phase * HD,
                                      n * 512:(n + 1) * 512]
                            nc.tensor.matmul(
                                ps[:], src, wos,
                                start=(phase == 0 and ki == 0),
                                stop=(phase == 2 and ki == 3))
                for bb, m, n, ps in chains:
                    ys = yo.tile([P, 512], F32, tag="ys", name="ys")
                    nc.vector.tensor_tensor(
                        ys[:], ps[:], bob[:, n * 512:(n + 1) * 512], ADD)
                    nc.sync.dma_start(
                        d_y[bb, m * P:(m + 1) * P, n * 512:(n + 1) * 512],
                        ys[:])

    nc.compile()
    return nc


def get_nc():
    if "nc" not in _CACHE:
        _CACHE["nc"] = _build_nc()
    return _CACHE["nc"]


def make_in_maps(x, Wq, bq, Wk, bk, Wv, bv, Wo, bo):
    bf16 = ml_dtypes.bfloat16
    x = np.asarray(x, dtype=np.float32)
    Wq, Wk, Wv, Wo = (np.asarray(w, dtype=np.float32) for w in (Wq, Wk, Wv, Wo))
    bq, bk, bv, bo = (np.asarray(v, dtype=np.float32) for v in (bq, bk, bv, bo))
    scale = 1.0 / np.sqrt(np.float32(HD))

    wo16 = np.ascontiguousarray(Wo.T).astype(bf16)
    bo1 = bo.reshape(1, D)

    in_maps = []
    for cc in range(CORES):
        b, g = cc // 4, cc % 4
        sl = slice(g * 256, (g + 1) * 256)
        x16 = np.ascontiguousarray(x[b].T).astype(bf16)
        wq16 = np.ascontiguousarray((Wq[sl, :] * scale).T).astype(bf16)
        wk16 = np.ascontiguousarray(Wk[sl, :].T).astype(bf16)
        wv16 = np.ascontiguousarray(Wv[sl, :].T).astype(bf16)
        pp = np.arange(P)
        bq2 = np.stack([bq[g * 256 + m * P + pp] * scale for m in range(2)],
                       axis=1).astype(np.float32)
        bk2 = np.stack([bk[g * 256 + m * P + pp] for m in range(2)],
                       axis=1).astype(np.float32)
        vib1 = bv[sl].reshape(1, 256).astype(np.float32)
        in_maps.append({
            "x16": x16, "wq16": wq16, "wk16": wk16, "wv16": wv16,
            "wo16": wo16, "bq2": np.ascontiguousarray(bq2),
            "bk2": np.ascontiguousarray(bk2), "vib1": vib1, "bo1": bo1,
        })
    return in_maps


def assemble(results):
    out = np.empty((B, S, D), dtype=np.float32)
    for c in range(CORES):
        out[:, c * QS:(c + 1) * QS, :] = results[c]["y"]
    return out


def kernel(**inputs):
    from concourse.bass_utils import run_bass_kernel_spmd

    nc = get_nc()
    in_maps = make_in_maps(**inputs)
    res = run_bass_kernel_spmd(nc, in_maps, list(range(CORES)), trace=False)
    return assemble(res.results)


# revision 46
# speedup vs baseline: 1.0046x; 1.0046x over previous
"""MHA (B=2, S=2048, D=1024, H=16) on 8 Trainium2 NeuronCores - bf16 edition.

Sharding: core c = (batch b = c//4, head-group g = c%4, 4 heads each).

All matmuls in bf16 (same PE rate as fp32r, half the DMA/SBUF traffic;
fp8 DoubleRow was tried and rejected: the sharp softmax rows amplify
any ~2% quantization noise past the accuracy gate). Structural wins
over the fp32r baseline:
 - softmax exp is split ACT(24/32, exact Exp) + DVE(8/32, Schraudolph
   fast-exp emitted as bf16 *bits* via a saturating fp32->uint16
   tensor_scalar), so the Activation engine is no longer the
   attention bottleneck; attention is PE-bound.
 - three AllToAlls (heads 0+1, head 2, head 3) in bf16 instead of four
   fp32 ones, mostly hidden under attention compute.
 - the out-projection runs in three waves (even k-tiles, then the
   head-2 and head-3 halves of the odd k-tiles as K=64 matmuls), each
   gated by one collective, so only the last wave sits on the tail.
 - scores PSUM is four 512-column banks deep so the PE never exposes
   the matmul->exp handoff latency; attention@V streams three
   key-blocks behind the exp wave.
"""

import numpy as np
import ml_dtypes

B, S, D, H = 2, 2048, 1024, 16
HD = D // H          # 64
GH = 4               # heads per core
CORES = 8
QS = S // CORES      # 256 per-core final sequence slice (per batch)
P = 128
KT = D // P          # 8 k-tiles of the model dim
VW = HD + 1          # 65: per-head V width incl. ones column

A16 = 128.0 / float(np.log(2.0))   # schraudolph slope (bf16-bit domain)
B16 = 16255.35                     # schraudolph offset (tuned)
LNC = 0.03638                      # ln(c): ACT path matches schraudolph scale

_CACHE = {}


def _exp_engine_pattern():
    """Weighted round-robin over (ACT, DVE) for the 64 exp tiles of one head.
    ACT-heavy keeps the approximate-exp share small; attention stays PE-bound
    either way."""
    shares = {"A": 42, "D": 22}
    counts = {k: 0 for k in shares}
    seq = []
    for i in range(64):
        pick = max(shares, key=lambda k: shares[k] * (i + 1) / 64 - counts[k])
        counts[pick] += 1
        seq.append(pick)
    return seq


def _build_nc():
    import concourse.mybir as mybir
    import concourse.tile as tile
    from concourse import bacc

    F32 = mybir.dt.float32
    BF16 = mybir.dt.bfloat16
    U16 = mybir.dt.uint16
    EXP = mybir.ActivationFunctionType.Exp
    IDN = mybir.ActivationFunctionType.Identity
    MUL = mybir.AluOpType.mult
    ADD = mybir.AluOpType.add

    nc = bacc.Bacc("TRN2", target_bir_lowering=False, debug=False,
                   num_devices=CORES)

    d_x = nc.dram_tensor("x16", [D, S], BF16, kind="ExternalInput")
    d_wq = nc.dram_tensor("wq16", [D, 256], BF16, kind="ExternalInput")
    d_wk = nc.dram_tensor("wk16", [D, 256], BF16, kind="ExternalInput")
    d_wv = nc.dram_tensor("wv16", [D, 256], BF16, kind="ExternalInput")
    d_wo = nc.dram_tensor("wo16", [D, D], BF16, kind="ExternalInput")
    d_bq = nc.dram_tensor("bq2", [P, 2], F32, kind="ExternalInput")
    d_bk = nc.dram_tensor("bk2", [P, 2], F32, kind="ExternalInput")
    d_vi = nc.dram_tensor("vib1", [1, 256], F32, kind="ExternalInput")
    d_bo = nc.dram_tensor("bo1", [1, D], F32, kind="ExternalInput")
    d_y = nc.dram_tensor("y", [B, QS, D], F32, kind="ExternalOutput")

    PAT = _exp_engine_pattern()

    with tile.TileContext(nc) as tc:
        with (
            tc.tile_pool(name="statics", bufs=1) as st,
            tc.tile_pool(name="dram", bufs=1, space="DRAM") as dram,
        ):
            bq = st.tile([P, 2], F32, tag="bq", name="bq")
            bk = st.tile([P, 2], F32, tag="bk", name="bk")
            lnc = st.tile([P, 1], F32, tag="lnc", name="lnc")
            vib = st.tile([P, 256], F32, tag="vib", name="vib")
            bob = st.tile([P, D], F32, tag="bob", name="bob")
            nc.vector.memset(lnc[:], LNC)

            # head pair tiles: partition = (h%2)*64 + hd  (PE base 0/64)
            qTp = [st.tile([P, S], BF16, tag=f"qT{m}", name=f"qT{m}")
                   for m in range(2)]
            kTp = [st.tile([P, S], BF16, tag=f"kT{m}", name=f"kT{m}")
                   for m in range(2)]
            vaug = [st.tile([P, GH * VW], BF16, tag=f"va{i}", name=f"va{i}")
                    for i in range(16)]
            pay = [st.tile([HD, S], BF16, tag=f"pay{h}", name=f"pay{h}")
                   for h in range(GH)]
            wo = [st.tile([P, D], BF16, tag=f"wo{k}", name=f"wo{k}")
                  for k in range(KT)]
            # merged A2A readback: aoE/aoO[:, (bb*4+gp)*256 + q] for even/odd
            # k-tiles; one DMA per a_out tensor instead of 24 small ones
            aoE = st.tile([P, 8 * QS], BF16, tag="aoE", name="aoE")
            aoO = st.tile([P, 8 * QS], BF16, tag="aoO", name="aoO")

            # ---- projections ----
            with (
                tc.tile_pool(name="proj", bufs=1) as pr,
                tc.tile_pool(name="pj", bufs=5, space="PSUM") as pj,
                tc.tile_pool(name="pv", bufs=2, space="PSUM") as pvp,
            ):
                wq16 = [pr.tile([P, 256], BF16, tag=f"wq{k}", name=f"wq{k}")
                        for k in range(KT)]
                wk16 = [pr.tile([P, 256], BF16, tag=f"wk{k}", name=f"wk{k}")
                        for k in range(KT)]
                wv16 = [pr.tile([P, 256], BF16, tag=f"wv{k}", name=f"wv{k}")
                        for k in range(KT)]
                xT = [pr.tile([P, S], BF16, tag=f"x{k}", name=f"x{k}")
                      for k in range(KT)]
                vi1 = pr.tile([1, 256], F32, tag="vi1", name="vi1")
                bo1 = pr.tile([1, D], F32, tag="bo1", name="bo1")
                # k-interleaved issue: chain step k waits only on DMAs
                # issued up to (wq,wk,x)[k], so the PE starts ~2us in
                for k in range(KT):
                    nc.sync.dma_start(wq16[k][:], d_wq[k * P:(k + 1) * P, :])
                    nc.sync.dma_start(wk16[k][:], d_wk[k * P:(k + 1) * P, :])
                    nc.sync.dma_start(xT[k][:], d_x[k * P:(k + 1) * P, :])
                nc.sync.dma_start(bq[:], d_bq[:])
                nc.sync.dma_start(bk[:], d_bk[:])
                for i in range(16):
                    ones = vaug[i].rearrange("p (h w) -> p h w", w=VW)
                    nc.vector.memset(ones[:, :, HD:VW], 1.0)

                # Q (bias-copy on ACT) / K (on DVE); k-major within groups of
                # chains so the PE streams behind the x DMA instead of
                # stalling on the full 4MB load; pair 0 first so head 0 can
                # start as soon as possible
                qk_chains = []
                for m in range(2):
                    for nb in range(4):
                        qk_chains.append(("K", m, nb))
                        qk_chains.append(("Q", m, nb))
                for g0 in range(0, 16, 4):
                    grp = qk_chains[g0:g0 + 4]
                    tiles = [pj.tile([P, 512], F32, tag="pj", name="pj")
                             for _ in grp]
                    for k in range(KT):
                        for (pk, m, nb), ps in zip(grp, tiles):
                            w = wk16[k] if pk == "K" else wq16[k]
                            nc.tensor.matmul(
                                ps[:], w[:, m * P:(m + 1) * P],
                                xT[k][:, nb * 512:(nb + 1) * 512],
                                start=(k == 0), stop=(k == KT - 1))
                    for (pk, m, nb), ps in zip(grp, tiles):
                        if pk == "K":
                            nc.vector.tensor_scalar(
                                kTp[m][:, nb * 512:(nb + 1) * 512], ps[:],
                                bk[:, m:m + 1], None, ADD)
                        else:
                            nc.scalar.activation(
                                qTp[m][:, nb * 512:(nb + 1) * 512], ps[:],
                                IDN, bias=bq[:, m:m + 1], scale=1.0)

                # V weights + biases now; wo last (needed only at out-proj)
                for k in range(KT):
                    nc.sync.dma_start(wv16[k][:], d_wv[k * P:(k + 1) * P, :])
                nc.sync.dma_start(vi1[:], d_vi[:])
                nc.sync.dma_start(bo1[:], d_bo[:])
                nc.gpsimd.partition_broadcast(vib[:], vi1[:])
                nc.gpsimd.partition_broadcast(bob[:], bo1[:])
                for k in range(KT):
                    nc.sync.dma_start(wo[k][:], d_wo[k * P:(k + 1) * P, :])

                # V: natural layout [kpos, 4 heads x 64] + ones col
                for sb in range(16):
                    pv = pvp.tile([P, 256], F32, tag="pv", name="pv")
                    for k in range(KT):
                        nc.tensor.matmul(
                            pv[:], xT[k][:, sb * P:(sb + 1) * P], wv16[k][:],
                            start=(k == 0), stop=(k == KT - 1))
                    dst = vaug[sb].rearrange("p (h w) -> p h w", w=VW)
                    nc.vector.tensor_tensor(
                        dst[:, :, 0:HD],
                        pv.rearrange("p (h w) -> p h w", w=HD),
                        vib.rearrange("p (h w) -> p h w", w=HD), ADD)

            # ---- attention ----
            a_ins = [dram.tile([CORES * P, QS], BF16, name="a_in01"),
                     dram.tile([CORES * HD, QS], BF16, name="a_in2"),
                     dram.tile([CORES * HD, QS], BF16, name="a_in3")]
            a_outs = [dram.tile([CORES * P, QS], BF16, name="a_out01"),
                      dram.tile([CORES * HD, QS], BF16, name="a_out2"),
                      dram.tile([CORES * HD, QS], BF16, name="a_out3")]

            def issue_collective(ci, heads):
                for hi, h in enumerate(heads):
                    dst = a_ins[ci].rearrange(
                        "(j r) q -> r j q", j=CORES)[hi * HD:(hi + 1) * HD]
                    src = pay[h].rearrange("p (j q) -> p j q", j=CORES)
                    nc.sync.dma_start(dst, src)
                nc.gpsimd.collective_compute(
                    "AllToAll",
                    mybir.AluOpType.bypass,
                    replica_groups=[list(range(CORES))],
                    ins=[a_ins[ci][:]],
                    outs=[a_outs[ci][:]],
                )

            with (
                tc.tile_pool(name="exp", bufs=1) as exp_pool,
                tc.tile_pool(name="nrm", bufs=2) as nr,
                tc.tile_pool(name="psc", bufs=4, space="PSUM") as psc,
                tc.tile_pool(name="pav", bufs=1, space="PSUM") as pav,
            ):
                for h in range(GH):
                    ksl = kTp[h // 2][(h % 2) * HD:(h % 2 + 1) * HD]
                    qsl = qTp[h // 2][(h % 2) * HD:(h % 2 + 1) * HD]
                    ex = [exp_pool.tile([P, S], BF16, tag=f"ex{i}",
                                        name=f"ex{i}") for i in range(16)]
                    av = pav.tile([VW, S], F32, tag="av", name="av")

                    def av_step(kb):
                        for qb in range(4):
                            nc.tensor.matmul(
                                av[:, qb * 512:(qb + 1) * 512],
                                vaug[kb][:, h * VW:(h + 1) * VW],
                                ex[kb][:, qb * 512:(qb + 1) * 512],
                                start=(kb == 0), stop=(kb == 15))

                    for kb in range(16):
                        # streaming AV three key-blocks behind the exp wave,
                        # issued BEFORE the scores pair so a scores psum-
                        # backpressure stall never head-of-line-blocks it
                        if kb >= 3:
                            av_step(kb - 3)
                        for qt in range(4):
                            sc = psc.tile([P, 512], F32, tag="sc", name="sc")
                            qo = qt * 512
                            nc.tensor.matmul(
                                sc[:], ksl[:, kb * P:(kb + 1) * P],
                                qsl[:, qo:qo + 512],
                                start=True, stop=True)
                            dst = ex[kb][:, qo:qo + 512]
                            if PAT[kb * 4 + qt] == "A":
                                nc.scalar.activation(
                                    dst, sc[:], EXP,
                                    bias=lnc[:, 0:1], scale=1.0)
                            else:
                                nc.vector.tensor_scalar(
                                    dst.bitcast(U16), sc[:],
                                    A16, B16, MUL, ADD)
                    for kb in range(13, 16):
                        av_step(kb)
                    rc = nr.tile([1, S], F32, tag="rc", name="rc")
                    nc.vector.reciprocal(rc[:], av[HD:VW, :])
                    rcb = nr.tile([HD, S], F32, tag="rcb", name="rcb")
                    nc.gpsimd.partition_broadcast(rcb[:], rc[:])
                    nc.vector.tensor_tensor(
                        pay[h][:], av[0:HD, :], rcb[:], MUL)

                    if h == 1:
                        issue_collective(0, [0, 1])
                    elif h == 2:
                        issue_collective(1, [2])
                        # even-k readback here: after this point SP.SEQ waits
                        # on pay[3]; issuing later would delay out-proj even
                        nc.sync.dma_start(
                            aoE.rearrange("p (s q) -> p s q", s=8),
                            a_outs[0].rearrange("(s p) q -> p s q", s=8))
                    elif h == 3:
                        issue_collective(2, [3])
                        # head-2 half of the odd-k readback: a_out2 lands
                        # while collective 3 is in flight
                        nc.sync.dma_start(
                            aoO.rearrange("p (s q) -> p s q", s=8)[0:HD],
                            a_outs[1].rearrange("(s p) q -> p s q", s=8))

            # head-3 half of the odd-k readback (after collective 3)
            nc.sync.dma_start(
                aoO.rearrange("p (s q) -> p s q", s=8)[HD:P],
                a_outs[2].rearrange("(s p) q -> p s q", s=8))

            # ---- out projection (my 256-row slice of each batch) ----
            with (
                tc.tile_pool(name="po", bufs=1, space="PSUM") as po,
                tc.tile_pool(name="yo", bufs=4) as yo,
            ):
                # create tiles in reverse so the first-executed chains sit
                # on the banks the attention scores pool releases earliest
                tiles = {}
                for key in [(bb, m, n) for bb in reversed(range(B))
                            for m in reversed(range(2))
                            for n in reversed(range(2))]:
                    tiles[key] = po.tile([P, 512], F32,
                                         tag="po{}{}{}".format(*key),
                                         name="po{}{}{}".format(*key))
                chains = [(bb, m, n, tiles[(bb, m, n)])
                          for bb in range(B) for m in range(2)
                          for n in range(2)]
                # three waves, each gated by one collective: even k-tiles
                # (heads 0,1), then the head-2 halves of the odd k-tiles
                # (K=64), then the head-3 halves after the last collective
                for phase in range(3):
                    for bb, m, n, ps in chains:
                        for ki in range(4):
                            k = ki * 2 + (1 if phase > 0 else 0)
                            c0 = (bb * 4 + ki) * QS + m * P
                            if phase == 0:
                                src = aoE[:, c0:c0 + P]
                            elif phase == 1:
                                src = aoO[0:HD, c0:c0 + P]
                            else:
                                src = aoO[HD:P, c0:c0 + P]
                            wos = wo[k][:, n * 512:(n + 1) * 512] \
                                if phase == 0 else \
                                wo[k][(phase - 1) * HD:phase * HD,
                                      n * 512:(n + 1) * 512]
                            nc.tensor.matmul(
                                ps[:], src, wos,
                                start=(phase == 0 and ki == 0),
                                stop=(phase == 2 and ki == 3))
                for bb, m, n, ps in chains:
                    ys = yo.tile([P, 512], F32, tag="ys", name="ys")
                    nc.vector.tensor_tensor(
                        ys[:], ps[:], bob[:, n * 512:(n + 1) * 512], ADD)
                    nc.sync.dma_start(
                        d_y[bb, m * P:(m + 1) * P, n * 512:(n + 1) * 512],
                        ys[:])

    nc.compile()
    return nc


def get_nc():
    if "nc" not in _CACHE:
        _CACHE["nc"] = _build_nc()
    return _CACHE["nc"]


def make_in_maps(x, Wq, bq, Wk, bk, Wv, bv, Wo, bo):
    bf16 = ml_dtypes.bfloat16
    x = np.asarray(x, dtype=np.float32)
    Wq, Wk, Wv, Wo = (np.asarray(w, dtype=np.float32) for w in (Wq, Wk, Wv, Wo))
    bq, bk, bv, bo = (np.asarray(v, dtype=np.float32) for v in (bq, bk, bv, bo))
    scale = 1.0 / np.sqrt(np.float32(HD))

    wo16 = np.ascontiguousarray(Wo.T).astype(bf16)
    bo1 = bo.reshape(1, D)

    in_maps = []
    for cc in range(CORES):
        b, g = cc // 4, cc % 4
        sl = slice(g * 256, (g + 1) * 256)
        x16 = np.ascontiguousarray(x[b].T).astype(bf16)
        wq16 = np.ascontiguousarray((Wq[sl, :] * scale).T).astype(bf16)
        wk16 = np.ascontiguousarray(Wk[sl, :].T).astype(bf16)
        wv16 = np.ascontiguousarray(Wv[sl, :].T).astype(bf16)
        pp = np.arange(P)
        bq2 = np.stack([bq[g * 256 + m * P + pp] * scale for m in range(2)],
                       axis=1).astype(np.float32)
        bk2 = np.stack([bk[g * 256 + m * P + pp] for m in range(2)],
                       axis=1).astype(np.float32)
        vib1 = bv[sl].reshape(1, 256).astype(np.float32)
        in_maps.append({
            "x16": x16, "wq16": wq16, "wk16": wk16, "wv16": wv16,
            "wo16": wo16, "bq2": np.ascontiguousarray(bq2),
            "bk2": np.ascontiguousarray(bk2), "vib1": vib1, "bo1": bo1,
        })
    return in_maps


def assemble(results):
    out = np.empty((B, S, D), dtype=np.float32)
    for c in range(CORES):
        out[:, c * QS:(c + 1) * QS, :] = results[c]["y"]
    return out


def kernel(**inputs):
    from concourse.bass_utils import run_bass_kernel_spmd

    nc = get_nc()
    in_maps = make_in_maps(**inputs)
    res = run_bass_kernel_spmd(nc, in_maps, list(range(CORES)), trace=False)
    return assemble(res.results)


# revision 47
# speedup vs baseline: 1.0079x; 1.0033x over previous
"""MHA (B=2, S=2048, D=1024, H=16) on 8 Trainium2 NeuronCores - bf16 edition.

Sharding: core c = (batch b = c//4, head-group g = c%4, 4 heads each).

All matmuls in bf16 (same PE rate as fp32r, half the DMA/SBUF traffic;
fp8 DoubleRow was tried and rejected: the sharp softmax rows amplify
any ~2% quantization noise past the accuracy gate). Structural wins
over the fp32r baseline:
 - softmax exp is split ACT(24/32, exact Exp) + DVE(8/32, Schraudolph
   fast-exp emitted as bf16 *bits* via a saturating fp32->uint16
   tensor_scalar), so the Activation engine is no longer the
   attention bottleneck; attention is PE-bound.
 - three AllToAlls (heads 0+1, head 2, head 3) in bf16 instead of four
   fp32 ones, mostly hidden under attention compute.
 - the out-projection runs in three waves (even k-tiles, then the
   head-2 and head-3 halves of the odd k-tiles as K=64 matmuls), each
   gated by one collective, so only the last wave sits on the tail.
 - scores PSUM is four 512-column banks deep so the PE never exposes
   the matmul->exp handoff latency; attention@V streams three
   key-blocks behind the exp wave.
"""

import numpy as np
import ml_dtypes

B, S, D, H = 2, 2048, 1024, 16
HD = D // H          # 64
GH = 4               # heads per core
CORES = 8
QS = S // CORES      # 256 per-core final sequence slice (per batch)
P = 128
KT = D // P          # 8 k-tiles of the model dim
VW = HD + 1          # 65: per-head V width incl. ones column

A16 = 128.0 / float(np.log(2.0))   # schraudolph slope (bf16-bit domain)
B16 = 16255.35                     # schraudolph offset (tuned)
LNC = 0.03638                      # ln(c): ACT path matches schraudolph scale

_CACHE = {}


def _exp_engine_pattern():
    """Weighted round-robin over (ACT, DVE) for the 64 exp tiles of one head.
    ACT-heavy keeps the approximate-exp share small; attention stays PE-bound
    either way."""
    shares = {"A": 42, "D": 22}
    counts = {k: 0 for k in shares}
    seq = []
    for i in range(64):
        pick = max(shares, key=lambda k: shares[k] * (i + 1) / 64 - counts[k])
        counts[pick] += 1
        seq.append(pick)
    return seq


def _build_nc():
    import concourse.mybir as mybir
    import concourse.tile as tile
    from concourse import bacc

    F32 = mybir.dt.float32
    BF16 = mybir.dt.bfloat16
    U16 = mybir.dt.uint16
    EXP = mybir.ActivationFunctionType.Exp
    IDN = mybir.ActivationFunctionType.Identity
    MUL = mybir.AluOpType.mult
    ADD = mybir.AluOpType.add

    nc = bacc.Bacc("TRN2", target_bir_lowering=False, debug=False,
                   num_devices=CORES)

    d_x = nc.dram_tensor("x16", [D, S], BF16, kind="ExternalInput")
    d_wq = nc.dram_tensor("wq16", [D, 256], BF16, kind="ExternalInput")
    d_wk = nc.dram_tensor("wk16", [D, 256], BF16, kind="ExternalInput")
    d_wv = nc.dram_tensor("wv16", [D, 256], BF16, kind="ExternalInput")
    d_wo = nc.dram_tensor("wo16", [D, D], BF16, kind="ExternalInput")
    d_bq = nc.dram_tensor("bq2", [P, 2], F32, kind="ExternalInput")
    d_bk = nc.dram_tensor("bk2", [P, 2], F32, kind="ExternalInput")
    d_vi = nc.dram_tensor("vib1", [1, 256], F32, kind="ExternalInput")
    d_bo = nc.dram_tensor("bo1", [1, D], F32, kind="ExternalInput")
    d_y = nc.dram_tensor("y", [B, QS, D], BF16, kind="ExternalOutput")

    PAT = _exp_engine_pattern()

    with tile.TileContext(nc) as tc:
        with (
            tc.tile_pool(name="statics", bufs=1) as st,
            tc.tile_pool(name="dram", bufs=1, space="DRAM") as dram,
        ):
            bq = st.tile([P, 2], F32, tag="bq", name="bq")
            bk = st.tile([P, 2], F32, tag="bk", name="bk")
            lnc = st.tile([P, 1], F32, tag="lnc", name="lnc")
            vib = st.tile([P, 256], F32, tag="vib", name="vib")
            bob = st.tile([P, D], F32, tag="bob", name="bob")
            nc.vector.memset(lnc[:], LNC)

            # head pair tiles: partition = (h%2)*64 + hd  (PE base 0/64)
            qTp = [st.tile([P, S], BF16, tag=f"qT{m}", name=f"qT{m}")
                   for m in range(2)]
            kTp = [st.tile([P, S], BF16, tag=f"kT{m}", name=f"kT{m}")
                   for m in range(2)]
            vaug = [st.tile([P, GH * VW], BF16, tag=f"va{i}", name=f"va{i}")
                    for i in range(16)]
            pay = [st.tile([HD, S], BF16, tag=f"pay{h}", name=f"pay{h}")
                   for h in range(GH)]
            wo = [st.tile([P, D], BF16, tag=f"wo{k}", name=f"wo{k}")
                  for k in range(KT)]
            # merged A2A readback: aoE/aoO[:, (bb*4+gp)*256 + q] for even/odd
            # k-tiles; one DMA per a_out tensor instead of 24 small ones
            aoE = st.tile([P, 8 * QS], BF16, tag="aoE", name="aoE")
            aoO = st.tile([P, 8 * QS], BF16, tag="aoO", name="aoO")

            # ---- projections ----
            with (
                tc.tile_pool(name="proj", bufs=1) as pr,
                tc.tile_pool(name="pj", bufs=5, space="PSUM") as pj,
                tc.tile_pool(name="pv", bufs=2, space="PSUM") as pvp,
            ):
                wq16 = [pr.tile([P, 256], BF16, tag=f"wq{k}", name=f"wq{k}")
                        for k in range(KT)]
                wk16 = [pr.tile([P, 256], BF16, tag=f"wk{k}", name=f"wk{k}")
                        for k in range(KT)]
                wv16 = [pr.tile([P, 256], BF16, tag=f"wv{k}", name=f"wv{k}")
                        for k in range(KT)]
                xT = [pr.tile([P, S], BF16, tag=f"x{k}", name=f"x{k}")
                      for k in range(KT)]
                vi1 = pr.tile([1, 256], F32, tag="vi1", name="vi1")
                bo1 = pr.tile([1, D], F32, tag="bo1", name="bo1")
                # k-interleaved issue: chain step k waits only on DMAs
                # issued up to (wq,wk,x)[k], so the PE starts ~2us in
                for k in range(KT):
                    nc.sync.dma_start(wq16[k][:], d_wq[k * P:(k + 1) * P, :])
                    nc.sync.dma_start(wk16[k][:], d_wk[k * P:(k + 1) * P, :])
                    nc.sync.dma_start(xT[k][:], d_x[k * P:(k + 1) * P, :])
                nc.sync.dma_start(bq[:], d_bq[:])
                nc.sync.dma_start(bk[:], d_bk[:])
                for i in range(16):
                    ones = vaug[i].rearrange("p (h w) -> p h w", w=VW)
                    nc.vector.memset(ones[:, :, HD:VW], 1.0)

                # Q (bias-copy on ACT) / K (on DVE); k-major within groups of
                # chains so the PE streams behind the x DMA instead of
                # stalling on the full 4MB load; pair 0 first so head 0 can
                # start as soon as possible
                qk_chains = []
                for m in range(2):
                    for nb in range(4):
                        qk_chains.append(("K", m, nb))
                        qk_chains.append(("Q", m, nb))
                for g0 in range(0, 16, 4):
                    grp = qk_chains[g0:g0 + 4]
                    tiles = [pj.tile([P, 512], F32, tag="pj", name="pj")
                             for _ in grp]
                    for k in range(KT):
                        for (pk, m, nb), ps in zip(grp, tiles):
                            w = wk16[k] if pk == "K" else wq16[k]
                            nc.tensor.matmul(
                                ps[:], w[:, m * P:(m + 1) * P],
                                xT[k][:, nb * 512:(nb + 1) * 512],
                                start=(k == 0), stop=(k == KT - 1))
                    for (pk, m, nb), ps in zip(grp, tiles):
                        if pk == "K":
                            nc.vector.tensor_scalar(
                                kTp[m][:, nb * 512:(nb + 1) * 512], ps[:],
                                bk[:, m:m + 1], None, ADD)
                        else:
                            nc.scalar.activation(
                                qTp[m][:, nb * 512:(nb + 1) * 512], ps[:],
                                IDN, bias=bq[:, m:m + 1], scale=1.0)

                # V weights + biases now; wo last (needed only at out-proj)
                for k in range(KT):
                    nc.sync.dma_start(wv16[k][:], d_wv[k * P:(k + 1) * P, :])
                nc.sync.dma_start(vi1[:], d_vi[:])
                nc.sync.dma_start(bo1[:], d_bo[:])
                nc.gpsimd.partition_broadcast(vib[:], vi1[:])
                nc.gpsimd.partition_broadcast(bob[:], bo1[:])
                for k in range(KT):
                    nc.sync.dma_start(wo[k][:], d_wo[k * P:(k + 1) * P, :])

                # V: natural layout [kpos, 4 heads x 64] + ones col
                for sb in range(16):
                    pv = pvp.tile([P, 256], F32, tag="pv", name="pv")
                    for k in range(KT):
                        nc.tensor.matmul(
                            pv[:], xT[k][:, sb * P:(sb + 1) * P], wv16[k][:],
                            start=(k == 0), stop=(k == KT - 1))
                    dst = vaug[sb].rearrange("p (h w) -> p h w", w=VW)
                    nc.vector.tensor_tensor(
                        dst[:, :, 0:HD],
                        pv.rearrange("p (h w) -> p h w", w=HD),
                        vib.rearrange("p (h w) -> p h w", w=HD), ADD)

            # ---- attention ----
            a_ins = [dram.tile([CORES * P, QS], BF16, name="a_in01"),
                     dram.tile([CORES * HD, QS], BF16, name="a_in2"),
                     dram.tile([CORES * HD, QS], BF16, name="a_in3")]
            a_outs = [dram.tile([CORES * P, QS], BF16, name="a_out01"),
                      dram.tile([CORES * HD, QS], BF16, name="a_out2"),
                      dram.tile([CORES * HD, QS], BF16, name="a_out3")]

            def issue_collective(ci, heads):
                for hi, h in enumerate(heads):
                    dst = a_ins[ci].rearrange(
                        "(j r) q -> r j q", j=CORES)[hi * HD:(hi + 1) * HD]
                    src = pay[h].rearrange("p (j q) -> p j q", j=CORES)
                    nc.sync.dma_start(dst, src)
                nc.gpsimd.collective_compute(
                    "AllToAll",
                    mybir.AluOpType.bypass,
                    replica_groups=[list(range(CORES))],
                    ins=[a_ins[ci][:]],
                    outs=[a_outs[ci][:]],
                )

            with (
                tc.tile_pool(name="exp", bufs=1) as exp_pool,
                tc.tile_pool(name="nrm", bufs=2) as nr,
                tc.tile_pool(name="psc", bufs=4, space="PSUM") as psc,
                tc.tile_pool(name="pav", bufs=1, space="PSUM") as pav,
            ):
                for h in range(GH):
                    ksl = kTp[h // 2][(h % 2) * HD:(h % 2 + 1) * HD]
                    qsl = qTp[h // 2][(h % 2) * HD:(h % 2 + 1) * HD]
                    ex = [exp_pool.tile([P, S], BF16, tag=f"ex{i}",
                                        name=f"ex{i}") for i in range(16)]
                    av = pav.tile([VW, S], F32, tag="av", name="av")

                    def av_step(kb):
                        for qb in range(4):
                            nc.tensor.matmul(
                                av[:, qb * 512:(qb + 1) * 512],
                                vaug[kb][:, h * VW:(h + 1) * VW],
                                ex[kb][:, qb * 512:(qb + 1) * 512],
                                start=(kb == 0), stop=(kb == 15))

                    for kb in range(16):
                        # streaming AV three key-blocks behind the exp wave,
                        # issued BEFORE the scores pair so a scores psum-
                        # backpressure stall never head-of-line-blocks it
                        if kb >= 3:
                            av_step(kb - 3)
                        for qt in range(4):
                            sc = psc.tile([P, 512], F32, tag="sc", name="sc")
                            qo = qt * 512
                            nc.tensor.matmul(
                                sc[:], ksl[:, kb * P:(kb + 1) * P],
                                qsl[:, qo:qo + 512],
                                start=True, stop=True)
                            dst = ex[kb][:, qo:qo + 512]
                            if PAT[kb * 4 + qt] == "A":
                                nc.scalar.activation(
                                    dst, sc[:], EXP,
                                    bias=lnc[:, 0:1], scale=1.0)
                            else:
                                nc.vector.tensor_scalar(
                                    dst.bitcast(U16), sc[:],
                                    A16, B16, MUL, ADD)
                    for kb in range(13, 16):
                        av_step(kb)
                    rc = nr.tile([1, S], F32, tag="rc", name="rc")
                    nc.vector.reciprocal(rc[:], av[HD:VW, :])
                    rcb = nr.tile([HD, S], F32, tag="rcb", name="rcb")
                    nc.gpsimd.partition_broadcast(rcb[:], rc[:])
                    nc.vector.tensor_tensor(
                        pay[h][:], av[0:HD, :], rcb[:], MUL)

                    if h == 1:
                        issue_collective(0, [0, 1])
                    elif h == 2:
                        issue_collective(1, [2])
                        # even-k readback here: after this point SP.SEQ waits
                        # on pay[3]; issuing later would delay out-proj even
                        nc.sync.dma_start(
                            aoE.rearrange("p (s q) -> p s q", s=8),
                            a_outs[0].rearrange("(s p) q -> p s q", s=8))
                    elif h == 3:
                        issue_collective(2, [3])
                        # head-2 half of the odd-k readback: a_out2 lands
                        # while collective 3 is in flight
                        nc.sync.dma_start(
                            aoO.rearrange("p (s q) -> p s q", s=8)[0:HD],
                            a_outs[1].rearrange("(s p) q -> p s q", s=8))

            # head-3 half of the odd-k readback (after collective 3)
            nc.sync.dma_start(
                aoO.rearrange("p (s q) -> p s q", s=8)[HD:P],
                a_outs[2].rearrange("(s p) q -> p s q", s=8))

            # ---- out projection (my 256-row slice of each batch) ----
            with (
                tc.tile_pool(name="po", bufs=1, space="PSUM") as po,
                tc.tile_pool(name="yo", bufs=4) as yo,
            ):
                # create tiles in reverse so the first-executed chains sit
                # on the banks the attention scores pool releases earliest
                tiles = {}
                for key in [(bb, m, n) for bb in reversed(range(B))
                            for m in reversed(range(2))
                            for n in reversed(range(2))]:
                    tiles[key] = po.tile([P, 512], F32,
                                         tag="po{}{}{}".format(*key),
                                         name="po{}{}{}".format(*key))
                chains = [(bb, m, n, tiles[(bb, m, n)])
                          for bb in range(B) for m in range(2)
                          for n in range(2)]
                # three waves, each gated by one collective: even k-tiles
                # (heads 0,1), then the head-2 halves of the odd k-tiles
                # (K=64), then the head-3 halves after the last collective
                for phase in range(3):
                    for bb, m, n, ps in chains:
                        for ki in range(4):
                            k = ki * 2 + (1 if phase > 0 else 0)
                            c0 = (bb * 4 + ki) * QS + m * P
                            if phase == 0:
                                src = aoE[:, c0:c0 + P]
                            elif phase == 1:
                                src = aoO[0:HD, c0:c0 + P]
                            else:
                                src = aoO[HD:P, c0:c0 + P]
                            wos = wo[k][:, n * 512:(n + 1) * 512] \
                                if phase == 0 else \
                                wo[k][(phase - 1) * HD:phase * HD,
                                      n * 512:(n + 1) * 512]
                            nc.tensor.matmul(
                                ps[:], src, wos,
                                start=(phase == 0 and ki == 0),
                                stop=(phase == 2 and ki == 3))
                for bb, m, n, ps in chains:
                    ys = yo.tile([P, 512], BF16, tag="ys", name="ys")
                    nc.vector.tensor_tensor(
                        ys[:], ps[:], bob[:, n * 512:(n + 1) * 512], ADD)
                    nc.sync.dma_start(
                        d_y[bb, m * P:(m + 1) * P, n * 512:(n + 1) * 512],
                        ys[:])

    nc.compile()
    return nc


def get_nc():
    if "nc" not in _CACHE:
        _CACHE["nc"] = _build_nc()
    return _CACHE["nc"]


def make_in_maps(x, Wq, bq, Wk, bk, Wv, bv, Wo, bo):
    bf16 = ml_dtypes.bfloat16
    x = np.asarray(x, dtype=np.float32)
    Wq, Wk, Wv, Wo = (np.asarray(w, dtype=np.float32) for w in (Wq, Wk, Wv, Wo))
    bq, bk, bv, bo = (np.asarray(v, dtype=np.float32) for v in (bq, bk, bv, bo))
    scale = 1.0 / np.sqrt(np.float32(HD))

    wo16 = np.ascontiguousarray(Wo.T).astype(bf16)
    bo1 = bo.reshape(1, D)

    in_maps = []
    for cc in range(CORES):
        b, g = cc // 4, cc % 4
        sl = slice(g * 256, (g + 1) * 256)
        x16 = np.ascontiguousarray(x[b].T).astype(bf16)
        wq16 = np.ascontiguousarray((Wq[sl, :] * scale).T).astype(bf16)
        wk16 = np.ascontiguousarray(Wk[sl, :].T).astype(bf16)
        wv16 = np.ascontiguousarray(Wv[sl, :].T).astype(bf16)
        pp = np.arange(P)
        bq2 = np.stack([bq[g * 256 + m * P + pp] * scale for m in range(2)],
                       axis=1).astype(np.float32)
        bk2 = np.stack([bk[g * 256 + m * P + pp] for m in range(2)],
                       axis=1).astype(np.float32)
        vib1 = bv[sl].reshape(1, 256).astype(np.float32)
        in_maps.append({
            "x16": x16, "wq16": wq16, "wk16": wk16, "wv16": wv16,
            "wo16": wo16, "bq2": np.ascontiguousarray(bq2),
            "bk2": np.ascontiguousarray(bk2), "vib1": vib1, "bo1": bo1,
        })
    return in_maps


def assemble(results):
    out = np.empty((B, S, D), dtype=np.float32)
    for c in range(CORES):
        out[:, c * QS:(c + 1) * QS, :] = np.asarray(
            results[c]["y"], dtype=np.float32)
    return out


def kernel(**inputs):
    from concourse.bass_utils import run_bass_kernel_spmd

    nc = get_nc()
    in_maps = make_in_maps(**inputs)
    res = run_bass_kernel_spmd(nc, in_maps, list(range(CORES)), trace=False)
    return assemble(res.results)
